# revision 21
# baseline (speedup 1.0000x reference)
"""Dual-stream multi-head attention on 8 Trainium2 NeuronCores (Bass/Tile).

Sharding: core c handles batch b = c//4 and head-group g = c%4 (4 of 16 heads).
Each core computes QKV projections (per-stream weights), RoPE, joint attention
over both streams, and a partial output projection (its heads' rows of wo).
The host sums the 4 per-core partials of each batch, transposes, and adds the
output bias.

On-chip layout is fully transposed ("feature dim on partitions, tokens on the
free dim"): x^T, q^T, k^T are [d, tokens]; scores are computed directly as
S^T = k_rope @ q_rope^T (k-positions on partitions), which lets the PV matmul
consume exp(S^T) with v in natural [token, dh] layout and produce o^T — the
exact layout the output projection wants.

Schedule notes (v3):
- All bulk tensors are stored partition-major in DRAM so DMAs move 2-16KB
  contiguous runs per partition (descriptor-efficient), chunked 4-ways so the
  PE can start consuming while the rest streams in.  Weight slabs and x tiles
  prefetch ahead of use (stream-1 slabs during tt=1, wo slabs in phase C).
- x^T is DMA'd ONCE per 512-token tile as a [128, 8192] slab shared by both
  head-pair QK passes and the V matmuls (stationary [128,128] column slices).
- RoPE's pair rotation runs on the DVE as a stream_shuffle (partition pair
  swap) with the signs folded into the host-prepared sin table; cos/sin
  multiplies and the combine run in bf16 (DVE 2x mode).  No PE or PSUM
  involvement beyond the projection itself.
- v bias is added by the DVE during the PSUM->SBUF move (host-replicated
  bias tile), freeing the scalar engine and the PE rank-1 matmul.
- exp() runs on [128,1024] tiles (two k-chunks per activation) to amortize
  activation-engine overhead; softmax denominators are chunk-accumulated in
  bf16 on the DVE (2x mode), folded, then row-summed with one ones-matmul.
- Output-projection chunks for query-tile qt-1 are interleaved into the
  attention j-loop of qt so the in-order PE queue always has ready matmuls
  while exp() catches up; out partials leave as bf16 pairs ([128,1024] DMAs)
  and the host sums partials in f32.
"""

import sys
import numpy as np

sys.path.insert(0, "/opt/trn_rl_repo")

import ml_dtypes
import concourse.bass as bass
import concourse.mybir as mybir
import concourse.tile as tile
from concourse.bass_utils import run_bass_kernel_spmd
from contextlib import ExitStack

B, N1, N2, D, H = 2, 1024, 1024, 2048, 16
T = N1 + N2              # 2048 tokens (both streams, concatenated)
DH = D // H              # 128
HPC = 4                  # heads per core
NKC = D // 128           # 16 contraction chunks
NTT = T // 512           # 4 512-token tiles
NTS = T // 128           # 16 128-token tiles
SCALE = DH ** -0.5
N_CORES = 8
SLAB = NKC * 512         # 8192 columns in a weight/x slab
CH = SLAB // 4           # 2048-column DMA chunks

BF = mybir.dt.bfloat16
F32 = mybir.dt.float32
F8 = mybir.dt.float8e4
bf16 = ml_dtypes.bfloat16
f8e4 = ml_dtypes.float8_e4m3
AF = mybir.ActivationFunctionType
ALU = mybir.AluOpType
DR = mybir.MatmulPerfMode.DoubleRow
SWAP_MASK = [i ^ 1 for i in range(32)]   # partition pair swap
WS = 32.0   # fp8 weight pre-scale: lifts the ~0.02-scale weights out of
            # e4m3's subnormal range; undone in the PSUM->SBUF activations

_BUILT = {}  # (repeats, phases) -> nc cache — build each program variant once


def build_program(repeats=1, phases="ABCD"):
    global _BUILT
    key = (repeats, phases)
    if key in _BUILT:
        return _BUILT[key]

    nc = bass.Bass()

    dts = {}
    for nm in ("xh", "xl"):
        dts[nm] = nc.dram_tensor(nm, [NTT, 128, SLAB], F8, kind="ExternalInput")
    for nm in ("wqh", "wql", "wkh", "wkl", "wvh", "wvl"):
        dts[nm] = nc.dram_tensor(nm, [2, 128, SLAB], F8, kind="ExternalInput")
    dts["wo"] = nc.dram_tensor("wo", [2, 128, HPC * D], BF, kind="ExternalInput")
    dts["bias_qk"] = nc.dram_tensor("bias_qk", [128, 16], F32, kind="ExternalInput")
    dts["bv"] = nc.dram_tensor("bv", [128, 2 * HPC * DH], BF, kind="ExternalInput")
    dts["cosT"] = nc.dram_tensor("cosT", [128, T], BF, kind="ExternalInput")
    dts["sinT"] = nc.dram_tensor("sinT", [128, T], BF, kind="ExternalInput")
    dts["outT"] = nc.dram_tensor("outT", [NTT, 128, SLAB], BF, kind="ExternalOutput")

    with tile.TileContext(nc) as tc:
        for _ in range(repeats):
            _emit(tc, nc, dts, phases=phases)

    _split_dma_waits(nc)
    _BUILT[key] = nc
    return nc


def _split_dma_waits(nc):
    """This walrus build's 64-byte instruction encoding holds exactly one sync
    wait and it does not auto-split ("Too many sync wait commands") when Tile
    assigns two or more.  Peel the extras into standalone EventSemaphore waits
    on the same engine immediately before the instruction."""
    wid = 0
    fn = nc.m.functions[0]
    for blk in fn.blocks:
        insts = blk.instructions
        out = []
        changed = False
        for inst in insts:
            si = inst.sync_info
            if si is not None and len(si.on_wait) > 1:
                waits = list(si.on_wait)
                for w in waits[:-1]:
                    pre = mybir.InstEventSemaphore(
                        name=f"WSPLIT-{wid}", ins=[], outs=[])
                    wid += 1
                    pre.engine = inst.engine
                    pre.sync_info = mybir.SyncInfo(on_wait=[w], on_update=[])
                    nc.register_instruction(pre, overwrite=True)
                    out.append(pre)
                inst.sync_info = mybir.SyncInfo(
                    on_wait=[waits[-1]], on_update=list(si.on_update))
                changed = True
            out.append(inst)
        if changed:
            blk.instructions = out


def _emit(tc, nc, dts, phases="ABCD"):
    wo_d, bias_d, bv_d = dts["wo"], dts["bias_qk"], dts["bv"]
    cos_d, sin_d, out_d = dts["cosT"], dts["sinT"], dts["outT"]
    with ExitStack() as top:
        consts = top.enter_context(tc.tile_pool(name="consts", bufs=1))
        persist = top.enter_context(tc.tile_pool(name="persist", bufs=1))

        bias_t = consts.tile([128, 16], F32, name="bias_t", tag="bias_t")
        nc.sync.dma_start(bias_t[:], bias_d[:])
        bv_t = consts.tile([128, 2 * HPC * DH], BF, name="bv_t", tag="bv_t")
        nc.sync.dma_start(bv_t[:], bv_d[:])
        cosT = consts.tile([128, T], BF, name="cosT_t", tag="cosT_t")
        sinT = consts.tile([128, T], BF, name="sinT_t", tag="sinT_t")
        ones_t = consts.tile([128, 128], BF, name="ones_t", tag="ones_t")
        nc.vector.memset(ones_t[:], 1.0)
        zero_t = consts.tile([128, 1], F32, name="zero_t", tag="zero_t")
        nc.vector.memset(zero_t[:], 0.0)

        q_rope = [persist.tile([128, T], BF, name=f"qrope{h}", tag=f"qrope{h}") for h in range(HPC)]
        k_rope = [persist.tile([128, T], BF, name=f"krope{h}", tag=f"krope{h}") for h in range(HPC)]
        v_sb = [persist.tile([128, HPC * DH], BF, name=f"vsb{ts}", tag=f"vsb{ts}") for ts in range(NTS)]

        # wv + x slabs live in the top scope: the v matmuls of the last token
        # tile are deferred into attention qt=0 to fill its exp-paced PE gaps.
        wv_pool = top.enter_context(tc.tile_pool(name="wvslab", bufs=2))
        xs_pool = top.enter_context(tc.tile_pool(name="xs", bufs=2))
        deferred_v = []          # (ts, s, slab-dict, xh, xl) emitted in phase C

        # ---------------- Phase A: q^T,k^T projections + RoPE --------------
        # ---------------- Phase B: v (natural layout) ----------------------
        with ExitStack() as ab:
            wslab = ab.enter_context(tc.tile_pool(name="wslab", bufs=2))
            tmp = ab.enter_context(tc.tile_pool(name="tmpab", bufs=3))
            qk_ps = ab.enter_context(tc.tile_pool(name="qkps", bufs=1, space="PSUM"))

            do_a = "A" in phases
            do_b = "B" in phases
            NJ = NKC // 2            # 8 kc-pairs per DoubleRow contraction

            def chunk(dst, src, c):
                nc.sync.dma_start(dst[:, c * CH:(c + 1) * CH], src[:, c * CH:(c + 1) * CH])

            def new_slabs(s):
                out = {}
                if do_a:
                    for nm in ("wqh", "wql", "wkh", "wkl"):
                        out[nm] = wslab.tile([128, SLAB], F8, name=f"{nm}{s}", tag=f"{nm}_slab")
                if do_b:
                    for nm in ("wvh", "wvl"):
                        out[nm] = wv_pool.tile([128, SLAB], F8, name=f"{nm}{s}", tag=f"{nm}_slab")
                return out

            def new_xslab(tt):
                return (xs_pool.tile([128, SLAB], F8, name=f"xh{tt}", tag="xslabh"),
                        xs_pool.tile([128, SLAB], F8, name=f"xl{tt}", tag="xslabl"))

            def pair3(t):
                """[128, SLAB] slab -> [128, NKC, 512] kc-major view."""
                return t[:].rearrange("p (kc f) -> p kc f", f=512)

            def alloc_qk_ps(pair, tt):
                hs = (2 * pair, 2 * pair + 1)
                qps, kps = {}, {}
                for h in hs:
                    qps[h] = qk_ps.tile([128, 512], F32, name=f"qps{tt}_{h}", tag=f"qk{h}q")
                    kps[h] = qk_ps.tile([128, 512], F32, name=f"kps{tt}_{h}", tag=f"qk{h}k")
                return qps, kps

            def emit_term_mms(pair, ti, sl, xh, xl, qps, kps):
                """One residual term (24/3=8 DR matmuls per psum group) of the
                q,k projections for one head pair."""
                hs = (2 * pair, 2 * pair + 1)
                wq_s, wk_s, xs = ((sl["wqh"], sl["wkh"], xh), (sl["wqh"], sl["wkh"], xl),
                                  (sl["wql"], sl["wkl"], xh))[ti]
                wq3, wk3, x3 = pair3(wq_s), pair3(wk_s), pair3(xs)
                for j in range(NJ):
                    xmv = x3[:, 2 * j:2 * j + 2, :]
                    for h in hs:
                        hsl2 = slice(h * DH, (h + 1) * DH)
                        nc.tensor.matmul(qps[h][:], wq3[:, 2 * j:2 * j + 2, hsl2], xmv,
                                         start=(ti == 0 and j == 0),
                                         stop=(ti == 2 and j == NJ - 1), perf_mode=DR)
                        nc.tensor.matmul(kps[h][:], wk3[:, 2 * j:2 * j + 2, hsl2], xmv,
                                         start=(ti == 0 and j == 0),
                                         stop=(ti == 2 and j == NJ - 1), perf_mode=DR)

            def emit_pair_mms(pair, tt, sl, xh, xl):
                qps, kps = alloc_qk_ps(pair, tt)
                for ti in range(3):
                    emit_term_mms(pair, ti, sl, xh, xl, qps, kps)
                return qps, kps

            def emit_rope(pair, tt, s, qps, kps):
                tsl = slice(tt * 512, (tt + 1) * 512)
                hs = (2 * pair, 2 * pair + 1)
                for h in hs:
                    for pj, (ps, dst) in enumerate(((qps[h], q_rope[h]), (kps[h], k_rope[h]))):
                        bj = s * 8 + pj * 4 + h
                        usc = (SCALE if pj == 0 else 1.0) / WS
                        sb = tmp.tile([128, 512], BF, name=f"sb{tt}{h}{pj}", tag="psb")
                        nc.scalar.activation(sb[:], ps[:], AF.Identity,
                                             bias=bias_t[:, bj:bj + 1], scale=usc)
                        rsb = tmp.tile([128, 512], BF, name=f"rs{tt}{h}{pj}", tag="rsb")
                        nc.vector.stream_shuffle(rsb[:], sb[:], SWAP_MASK)
                        t1 = tmp.tile([128, 512], BF, name=f"t1_{tt}{h}{pj}", tag="t1")
                        nc.vector.tensor_tensor(t1[:], sb[:], cosT[:, tsl], ALU.mult)
                        t2 = tmp.tile([128, 512], BF, name=f"t2_{tt}{h}{pj}", tag="t2")
                        nc.vector.tensor_tensor(t2[:], rsb[:], sinT[:, tsl], ALU.mult)
                        nc.vector.tensor_tensor(dst[:, tsl], t1[:], t2[:], ALU.add)

            def emit_v(ts, s, sl, xh, xl, ps_pool=None, ps_tag="vps"):
                # v for token-slice ts (128 tokens) from the cached x slabs:
                # stationary = x^T column chunk pair, moving = wv slab pair.
                jj = ts % 4
                csl = slice(jj * 128, (jj + 1) * 128)
                vps = (ps_pool or v_ps).tile([128, 512], F32, name=f"vp{ts}", tag=ps_tag)
                terms = ((xh, sl["wvh"]), (xl, sl["wvh"]), (xh, sl["wvl"]))
                for ti, (xs, wv_s) in enumerate(terms):
                    x3, wv3 = pair3(xs), pair3(wv_s)
                    for j in range(NJ):
                        nc.tensor.matmul(vps[:], x3[:, 2 * j:2 * j + 2, csl],
                                         wv3[:, 2 * j:2 * j + 2, :],
                                         start=(ti == 0 and j == 0),
                                         stop=(ti == 2 and j == NJ - 1), perf_mode=DR)
                nc.vector.scalar_tensor_tensor(v_sb[ts][:], vps[:], 1.0 / WS,
                                               bv_t[:, s * 512:(s + 1) * 512],
                                               ALU.mult, ALU.add)

            if do_a or do_b:
                # Prologue: stream-0 slabs + x(tt=0), chunk-interleaved in
                # needed-first order; cos/sin after the first chunk group.
                slabs = {0: new_slabs(0)}
                xh, xl = new_xslab(0)
                for c in range(4):
                    if do_a:
                        chunk(slabs[0]["wqh"], dts["wqh"][0], c)
                        chunk(slabs[0]["wkh"], dts["wkh"][0], c)
                    chunk(xh, dts["xh"][0], c)
                    chunk(xl, dts["xl"][0], c)
                    if do_a:
                        chunk(slabs[0]["wql"], dts["wql"][0], c)
                        chunk(slabs[0]["wkl"], dts["wkl"][0], c)
                    if c == 0:
                        nc.sync.dma_start(cosT[:], cos_d[:])
                        nc.sync.dma_start(sinT[:], sin_d[:])
                    elif do_b:
                        chunk(slabs[0]["wvh"], dts["wvh"][0], c - 1)
                if do_b:
                    chunk(slabs[0]["wvh"], dts["wvh"][0], 3)
                    for c in range(4):
                        chunk(slabs[0]["wvl"], dts["wvl"][0], c)

                for tt in range(NTT):
                    s = tt // 2
                    if tt > 0:
                        xh, xl = new_xslab(tt)
                        for c in range(4):
                            chunk(xh, dts["xh"][tt], c)
                            chunk(xl, dts["xl"][tt], c)
                    if tt == 1:
                        slabs[1] = new_slabs(1)
                        for c in range(4):
                            for nm in slabs[1]:
                                chunk(slabs[1][nm], dts[nm][1], c)
                    sl = slabs[s]
                    defer = do_b and tt == NTT - 1 and "C" in phases
                    if defer:
                        for u in range(4):
                            deferred_v.append((4 * tt + u, s, sl, xh, xl, emit_v))
                    if do_a:
                        qps0, kps0 = emit_pair_mms(0, tt, sl, xh, xl)
                    if do_b and not defer:
                        emit_v(4 * tt + 0, s, sl, xh, xl)
                    if do_a:
                        qps1, kps1 = emit_pair_mms(1, tt, sl, xh, xl)
                        emit_rope(0, tt, s, qps0, kps0)
                    if do_b and not defer:
                        emit_v(4 * tt + 1, s, sl, xh, xl)
                        emit_v(4 * tt + 2, s, sl, xh, xl)
                    if do_a:
                        emit_rope(1, tt, s, qps1, kps1)
                    if do_b and not defer:
                        emit_v(4 * tt + 3, s, sl, xh, xl)

        # ------- Phase C+D: attention + output projection, interleaved -----
        with ExitStack() as att:
            sps_ps = att.enter_context(tc.tile_pool(name="spsps", bufs=2, space="PSUM"))
            oacc_ps = att.enter_context(tc.tile_pool(name="oaccps", bufs=1, space="PSUM"))
            sums_ps_pool = att.enter_context(tc.tile_pool(name="sumsps", bufs=1, space="PSUM"))
            out_ps = att.enter_context(tc.tile_pool(name="outps", bufs=2, space="PSUM"))
            es_pool = att.enter_context(tc.tile_pool(name="es", bufs=4))
            sacc_pool = att.enter_context(tc.tile_pool(name="sacc", bufs=2))
            rc_pool = att.enter_context(tc.tile_pool(name="rc", bufs=2))
            wo_pool = att.enter_context(tc.tile_pool(name="wopool", bufs=2))
            osb_pool = att.enter_context(tc.tile_pool(name="osb", bufs=4))
            onorm_pool = att.enter_context(tc.tile_pool(name="onorm", bufs=1))

            o_norm = [onorm_pool.tile([128, T], BF, name=f"onorm{h}", tag=f"onorm{h}")
                      for h in range(HPC)]
            wo_slabs = {}
            osb_cur = [None]

            def issue_wo(s):
                wo_slab = wo_pool.tile([128, HPC * D], BF, name=f"wos{s}", tag="wo_slab")
                for c in range(4):
                    nc.sync.dma_start(wo_slab[:, c * CH:(c + 1) * CH],
                                      wo_d[s][:, c * CH:(c + 1) * CH])
                wo_slabs[s] = wo_slab

            def emit_outproj_chunk(qt, od, tail=False):
                qsl = slice(qt * 512, (qt + 1) * 512)
                wo_slab = wo_slabs[qt // 2]
                ops_t = out_ps.tile([128, 512], F32, name=f"op{qt}{od}", tag="o")
                for hd in range(HPC):
                    nc.tensor.matmul(
                        ops_t[:], wo_slab[:, hd * D + od * 128: hd * D + (od + 1) * 128],
                        o_norm[hd][:, qsl], start=(hd == 0), stop=(hd == HPC - 1))
                if od % 2 == 0:
                    osb_cur[0] = osb_pool.tile([128, 1024], BF, name=f"ou{qt}{od}", tag="osb")
                half = od % 2
                if tail:
                    # the final query tile drains after exp() is done — use the
                    # idle scalar engine so the DVE doesn't become the tail
                    # bottleneck
                    nc.scalar.activation(osb_cur[0][:, half * 512:(half + 1) * 512],
                                         ops_t[:], AF.Copy)
                else:
                    nc.vector.tensor_copy(osb_cur[0][:, half * 512:(half + 1) * 512], ops_t[:])
                if od % 2 == 1:
                    nc.sync.dma_start(out_d[qt][:, (od - 1) * 512:(od + 1) * 512], osb_cur[0][:])

            if "C" in phases:
                do_d = "D" in phases
                if do_d:
                    issue_wo(0)
                for qt in range(NTT):
                    qsl = slice(qt * 512, (qt + 1) * 512)
                    if qt == 2 and do_d:
                        issue_wo(1)
                    for h in range(HPC):
                        oacc = oacc_ps.tile([128, 512], F32, name=f"oa{h}{qt}", tag="oacc")
                        sacc = sacc_pool.tile([128, 1024], BF, name=f"sa{h}{qt}", tag="sacc")
                        es_prev = None
                        for j in range(NKC // 2):
                            sps = sps_ps.tile([128, 1024], F32, name=f"sp{h}{qt}{j}", tag="sps")
                            for half in range(2):
                                kc = 2 * j + half
                                nc.tensor.matmul(sps[:, half * 512:(half + 1) * 512],
                                                 k_rope[h][:, kc * 128:(kc + 1) * 128],
                                                 q_rope[h][:, qsl], start=True, stop=True)
                            es = es_pool.tile([128, 1024], BF, name=f"es{h}{qt}{j}", tag="es")
                            nc.scalar.activation(es[:], sps[:], AF.Exp, bias=zero_t[:, 0:1])
                            for half in range(2):
                                kc = 2 * j + half
                                nc.tensor.matmul(oacc[:], v_sb[kc][:, h * DH:(h + 1) * DH],
                                                 es[:, half * 512:(half + 1) * 512],
                                                 start=(kc == 0), stop=(kc == NKC - 1))
                            if j == 1:
                                nc.vector.tensor_tensor(sacc[:], es_prev[:], es[:], ALU.add)
                            elif j > 1:
                                nc.vector.tensor_tensor(sacc[:], sacc[:], es[:], ALU.add)
                            es_prev = es
                            # Interleave one ready output-projection chunk of the
                            # previous query tile after every odd j: 4 per head x
                            # 4 heads = all 16 chunks of qt-1.  At qt=0 there is
                            # no projection work yet — the deferred last-token-
                            # tile v units fill the exp-paced PE gaps instead
                            # (they must all land before j=6 consumes v_sb[12:]).
                            if do_d and qt > 0 and j % 2 == 1:
                                emit_outproj_chunk(qt - 1, h * 4 + j // 2)
                            if qt == 0 and h == 0 and j < len(deferred_v):
                                ts, s_, sl_, xh_, xl_, ev = deferred_v[j]
                                ev(ts, s_, sl_, xh_, xl_, ps_pool=out_ps, ps_tag="o")
                        sfold = sacc_pool.tile([128, 512], BF, name=f"sf{h}{qt}", tag="sfold")
                        nc.vector.tensor_tensor(sfold[:], sacc[:, 0:512], sacc[:, 512:1024], ALU.add)
                        sums = sums_ps_pool.tile([128, 512], F32, name=f"su{h}{qt}", tag="sums")
                        nc.tensor.matmul(sums[:], ones_t[:], sfold[:], start=True, stop=True)
                        rc = rc_pool.tile([128, 512], F32, name=f"rc{h}{qt}", tag="rc")
                        nc.vector.reciprocal(rc[:], sums[:])
                        nc.vector.tensor_tensor(o_norm[h][:, qsl], oacc[:], rc[:], ALU.mult)
                if do_d:
                    for od in range(NKC):
                        emit_outproj_chunk(NTT - 1, od, tail=True)


def shard_inputs(inputs):
    """Full inputs -> per-core in_maps (all host-side prep: transpose, cast,
    scale-folding, per-head slicing, partition-major repacks)."""
    f32 = np.float32
    x1, x2 = np.asarray(inputs["x_1"], f32), np.asarray(inputs["x_2"], f32)
    cosT = np.ascontiguousarray(
        np.concatenate([np.asarray(inputs["cos1"]), np.asarray(inputs["cos2"])], 0).T
    ).astype(bf16)
    sinT = np.concatenate([np.asarray(inputs["sin1"]), np.asarray(inputs["sin2"])], 0).T.copy()
    sinT[0::2, :] *= -1.0      # fold the pair-rotation signs into sin
    sinT = np.ascontiguousarray(sinT).astype(bf16)

    def hilo(x):
        hi = x.astype(f8e4)
        lo = (x - hi.astype(f32)).astype(f8e4)
        return hi, lo

    in_maps = []
    for c in range(N_CORES):
        b, hg = divmod(c, 4)
        hsl = slice(hg * HPC * DH, (hg + 1) * HPC * DH)
        xc = np.concatenate([x1[b], x2[b]], 0)          # [T, D]
        xT = xc.T.reshape(NKC, 128, NTT, 512)
        xTT = np.ascontiguousarray(xT.transpose(2, 1, 0, 3).reshape(NTT, 128, SLAB))
        xh, xl = hilo(xTT)

        def wslice(name):
            """Weight slab pair, pre-scaled by WS into fp8 hi+lo."""
            hi = np.empty((2, 128, SLAB), f8e4)
            lo = np.empty((2, 128, SLAB), f8e4)
            for s in range(2):
                w = np.asarray(inputs[name + str(s + 1)], f32)[:, hsl] * WS
                w = w.reshape(NKC, 128, HPC * DH).transpose(1, 0, 2).reshape(128, SLAB)
                hi[s], lo[s] = hilo(np.ascontiguousarray(w))
            return hi, lo

        wqh, wql = wslice("wq")
        wkh, wkl = wslice("wk")
        wvh, wvl = wslice("wv")
        wo = np.empty((2, 128, HPC * D), bf16)
        for s in range(2):
            w = np.asarray(inputs["wo" + str(s + 1)], f32)[hsl, :]
            wo[s] = w.reshape(HPC, 128, D).transpose(1, 0, 2).reshape(128, HPC * D).astype(bf16)

        bias = np.zeros((128, 16), f32)
        for s in range(2):
            bqs = np.asarray(inputs["bq" + str(s + 1)], f32)[hsl] * SCALE
            bks = np.asarray(inputs["bk" + str(s + 1)], f32)[hsl]
            for h in range(HPC):
                bias[:, s * 8 + h] = bqs[h * DH:(h + 1) * DH]
                bias[:, s * 8 + 4 + h] = bks[h * DH:(h + 1) * DH]
        bv = np.concatenate([
            np.asarray(inputs["bv1"], f32)[hsl], np.asarray(inputs["bv2"], f32)[hsl]
        ]).reshape(1, 2 * HPC * DH)
        bv = np.ascontiguousarray(np.broadcast_to(bv, (128, 2 * HPC * DH))).astype(bf16)

        in_maps.append({
            "xh": xh, "xl": xl, "wqh": wqh, "wql": wql, "wkh": wkh, "wkl": wkl,
            "wvh": wvh, "wvl": wvl, "wo": wo,
            "bias_qk": bias, "bv": bv, "cosT": cosT, "sinT": sinT,
        })
    return in_maps


def unshard_outputs(results, inputs):
    f32 = np.float32
    acc = np.zeros((B, D, T), f32)
    for c in range(N_CORES):
        r = results[c]["outT"].astype(f32)               # [NTT, 128, NKC*512]
        acc[c // 4] += r.reshape(NTT, 128, NKC, 512).transpose(2, 1, 0, 3).reshape(D, T)
    o1 = np.empty((B, N1, D), f32)
    o2 = np.empty((B, N2, D), f32)
    bo1 = np.asarray(inputs["bo1"], f32)
    bo2 = np.asarray(inputs["bo2"], f32)
    for b in range(B):
        full = acc[b].T                                  # [T, D]
        o1[b] = full[:N1] + bo1
        o2[b] = full[N1:] + bo2
    return o1, o2


def kernel(**inputs):
    nc = build_program()
    in_maps = shard_inputs(inputs)
    res = run_bass_kernel_spmd(nc, in_maps, list(range(N_CORES)))
    return unshard_outputs(res.results, inputs)


if __name__ == "__main__":
    data = np.load("/root/problem/cache_inputs.npz")
    out = kernel(**{k: data[k] for k in data.files})
    exp = np.load("/root/problem/cache_expected.npz")
    for i, o in enumerate(out):
        e = exp[f"o{i+1}"]
        d = np.abs(o - e).max()
        print(f"o{i+1}: absmax_err {d:.4e} rel {d / np.abs(e).max():.4e}")


# revision 33
# speedup vs baseline: 1.3371x; 1.3371x over previous
"""Dual-stream multi-head attention on 8 Trainium2 NeuronCores (Bass/Tile).

Sharding: core c handles batch b = c//4 and head-group g = c%4 (4 of 16 heads).
Each core computes QKV projections (per-stream weights), RoPE, joint attention
over both streams, and a partial output projection (its heads' rows of wo).
The host sums the 4 per-core partials of each batch, transposes, and adds the
output bias.

On-chip layout is fully transposed ("feature dim on partitions, tokens on the
free dim"): x^T, q^T, k^T are [d, tokens]; scores are computed directly as
S^T = k_rope @ q_rope^T (k-positions on partitions), which lets the PV matmul
consume exp(S^T) with v in natural [token, dh] layout and produce o^T — the
exact layout the output projection wants.

Schedule notes (v3):
- All bulk tensors are stored partition-major in DRAM so DMAs move 2-16KB
  contiguous runs per partition (descriptor-efficient), chunked 4-ways so the
  PE can start consuming while the rest streams in.  Weight slabs and x tiles
  prefetch ahead of use (stream-1 slabs during tt=1, wo slabs in phase C).
- x^T is DMA'd ONCE per 512-token tile as a [128, 8192] slab shared by both
  head-pair QK passes and the V matmuls (stationary [128,128] column slices).
- RoPE's pair rotation runs on the DVE as a stream_shuffle (partition pair
  swap) with the signs folded into the host-prepared sin table; cos/sin
  multiplies and the combine run in bf16 (DVE 2x mode).  No PE or PSUM
  involvement beyond the projection itself.
- v bias is added by the DVE during the PSUM->SBUF move (host-replicated
  bias tile), freeing the scalar engine and the PE rank-1 matmul.
- exp() runs on [128,1024] tiles (two k-chunks per activation) to amortize
  activation-engine overhead; softmax denominators are chunk-accumulated in
  bf16 on the DVE (2x mode), folded, then row-summed with one ones-matmul.
- Output-projection chunks for query-tile qt-1 are interleaved into the
  attention j-loop of qt so the in-order PE queue always has ready matmuls
  while exp() catches up; out partials leave as bf16 pairs ([128,1024] DMAs)
  and the host sums partials in f32.
"""

import sys
import numpy as np

sys.path.insert(0, "/opt/trn_rl_repo")

import ml_dtypes
import concourse.bass as bass
import concourse.mybir as mybir
import concourse.tile as tile
from concourse.bass_utils import run_bass_kernel_spmd
from contextlib import ExitStack

B, N1, N2, D, H = 2, 1024, 1024, 2048, 16
T = N1 + N2              # 2048 tokens (both streams, concatenated)
DH = D // H              # 128
HPC = 4                  # heads per core
NKC = D // 128           # 16 contraction chunks
NTT = T // 512           # 4 512-token tiles
NTS = T // 128           # 16 128-token tiles
SCALE = DH ** -0.5
N_CORES = 8
SLAB = NKC * 512         # 8192 columns in a weight/x slab
CH = SLAB // 4           # 2048-column DMA chunks

BF = mybir.dt.bfloat16
F32 = mybir.dt.float32
F8 = mybir.dt.float8e4
bf16 = ml_dtypes.bfloat16
f8e4 = ml_dtypes.float8_e4m3
AF = mybir.ActivationFunctionType
ALU = mybir.AluOpType
DR = mybir.MatmulPerfMode.DoubleRow
SWAP_MASK = [i ^ 1 for i in range(32)]   # partition pair swap
WS = 32.0   # fp8 weight pre-scale: lifts the ~0.02-scale weights out of
            # e4m3's subnormal range; undone in the PSUM->SBUF activations

_BUILT = {}  # (repeats, phases) -> nc cache — build each program variant once


def build_program(repeats=1, phases="ABCD"):
    global _BUILT
    key = (repeats, phases)
    if key in _BUILT:
        return _BUILT[key]

    nc = bass.Bass()

    dts = {}
    dts["xT"] = nc.dram_tensor("xT", [NTT, 128, SLAB], BF, kind="ExternalInput")
    for nm in ("wq", "wk", "wv"):
        dts[nm] = nc.dram_tensor(nm, [2, 128, SLAB], BF, kind="ExternalInput")
    dts["wo"] = nc.dram_tensor("wo", [2, 128, HPC * D], BF, kind="ExternalInput")
    dts["bias_qk"] = nc.dram_tensor("bias_qk", [128, 16], F32, kind="ExternalInput")
    dts["bv"] = nc.dram_tensor("bv", [128, 2 * HPC * DH], BF, kind="ExternalInput")
    dts["cosT"] = nc.dram_tensor("cosT", [128, T], BF, kind="ExternalInput")
    dts["sinT"] = nc.dram_tensor("sinT", [128, T], BF, kind="ExternalInput")
    dts["outT"] = nc.dram_tensor("outT", [NTT, 128, SLAB], BF, kind="ExternalOutput")

    with tile.TileContext(nc) as tc:
        for _ in range(repeats):
            _emit(tc, nc, dts, phases=phases)

    _split_dma_waits(nc)
    _BUILT[key] = nc
    return nc


def _split_dma_waits(nc):
    """This walrus build's 64-byte instruction encoding holds exactly one sync
    wait and it does not auto-split ("Too many sync wait commands") when Tile
    assigns two or more.  Peel the extras into standalone EventSemaphore waits
    on the same engine immediately before the instruction."""
    wid = 0
    fn = nc.m.functions[0]
    for blk in fn.blocks:
        insts = blk.instructions
        out = []
        changed = False
        for inst in insts:
            si = inst.sync_info
            if si is not None and len(si.on_wait) > 1:
                waits = list(si.on_wait)
                for w in waits[:-1]:
                    pre = mybir.InstEventSemaphore(
                        name=f"WSPLIT-{wid}", ins=[], outs=[])
                    wid += 1
                    pre.engine = inst.engine
                    pre.sync_info = mybir.SyncInfo(on_wait=[w], on_update=[])
                    nc.register_instruction(pre, overwrite=True)
                    out.append(pre)
                inst.sync_info = mybir.SyncInfo(
                    on_wait=[waits[-1]], on_update=list(si.on_update))
                changed = True
            out.append(inst)
        if changed:
            blk.instructions = out


def _emit(tc, nc, dts, phases="ABCD"):
    wo_d, bias_d, bv_d = dts["wo"], dts["bias_qk"], dts["bv"]
    cos_d, sin_d, out_d = dts["cosT"], dts["sinT"], dts["outT"]
    with ExitStack() as top:
        consts = top.enter_context(tc.tile_pool(name="consts", bufs=1))
        persist = top.enter_context(tc.tile_pool(name="persist", bufs=1))

        bias_t = consts.tile([128, 16], F32, name="bias_t", tag="bias_t")
        bv_t = consts.tile([128, 2 * HPC * DH], BF, name="bv_t", tag="bv_t")
        cosT = consts.tile([128, T], BF, name="cosT_t", tag="cosT_t")
        sinT = consts.tile([128, T], BF, name="sinT_t", tag="sinT_t")
        ones_t = consts.tile([128, 128], BF, name="ones_t", tag="ones_t")
        nc.vector.memset(ones_t[:], 1.0)
        zero_t = consts.tile([128, 1], F32, name="zero_t", tag="zero_t")
        nc.vector.memset(zero_t[:], 0.0)

        q_rope = [persist.tile([128, T], BF, name=f"qrope{h}", tag=f"qrope{h}") for h in range(HPC)]
        k_rope = [persist.tile([128, T], BF, name=f"krope{h}", tag=f"krope{h}") for h in range(HPC)]
        v_sb = [persist.tile([128, HPC * DH], BF, name=f"vsb{ts}", tag=f"vsb{ts}") for ts in range(NTS)]

        # wv + x slabs live in the top scope: the v matmuls of the last token
        # tile are deferred into attention qt=0 to fill its exp-paced PE gaps.
        wv_pool = top.enter_context(tc.tile_pool(name="wvslab", bufs=2))
        xs_pool = top.enter_context(tc.tile_pool(name="xs", bufs=2))
        deferred_v = []          # (ts, s, slab-dict, xh, xl) emitted in phase C

        # ---------------- Phase A: q^T,k^T projections + RoPE --------------
        # ---------------- Phase B: v (natural layout) ----------------------
        with ExitStack() as ab:
            wslab = ab.enter_context(tc.tile_pool(name="wslab", bufs=2))
            tmp = ab.enter_context(tc.tile_pool(name="tmpab", bufs=3))
            qk_ps = ab.enter_context(tc.tile_pool(name="qkps", bufs=1, space="PSUM"))

            do_a = "A" in phases
            do_b = "B" in phases

            def chunk(dst, src, c):
                nc.sync.dma_start(dst[:, c * CH:(c + 1) * CH], src[:, c * CH:(c + 1) * CH])

            def new_slabs(s):
                out = {}
                if do_a:
                    for nm in ("wq", "wk"):
                        out[nm] = wslab.tile([128, SLAB], BF, name=f"{nm}{s}", tag=f"{nm}_slab")
                if do_b:
                    out["wv"] = wv_pool.tile([128, SLAB], BF, name=f"wv{s}", tag="wv_slab")
                return out

            def new_xslab(tt):
                return xs_pool.tile([128, SLAB], BF, name=f"x{tt}", tag="xslab")

            def emit_pair_mms(pair, tt, sl, xs):
                """One head pair's q,k projections, kc-major (DMA arrival
                order at tt=0)."""
                hs = (2 * pair, 2 * pair + 1)
                qps, kps = {}, {}
                for h in hs:
                    qps[h] = qk_ps.tile([128, 512], F32, name=f"qps{tt}_{h}", tag=f"qk{h}q")
                    kps[h] = qk_ps.tile([128, 512], F32, name=f"kps{tt}_{h}", tag=f"qk{h}k")
                for kc in range(NKC):
                    xsl = xs[:, kc * 512:(kc + 1) * 512]
                    for h in hs:
                        wsl = slice(kc * 512 + h * DH, kc * 512 + (h + 1) * DH)
                        nc.tensor.matmul(qps[h][:], sl["wq"][:, wsl], xsl,
                                         start=(kc == 0), stop=(kc == NKC - 1))
                        nc.tensor.matmul(kps[h][:], sl["wk"][:, wsl], xsl,
                                         start=(kc == 0), stop=(kc == NKC - 1))
                return qps, kps

            def emit_rope(pair, tt, s, qps, kps):
                tsl = slice(tt * 512, (tt + 1) * 512)
                hs = (2 * pair, 2 * pair + 1)
                for h in hs:
                    for pj, (ps, dst) in enumerate(((qps[h], q_rope[h]), (kps[h], k_rope[h]))):
                        bj = s * 8 + pj * 4 + h
                        sb = tmp.tile([128, 512], BF, name=f"sb{tt}{h}{pj}", tag="psb")
                        nc.scalar.activation(sb[:], ps[:], AF.Identity,
                                             bias=bias_t[:, bj:bj + 1])
                        rsb = tmp.tile([128, 512], BF, name=f"rs{tt}{h}{pj}", tag="rsb")
                        nc.vector.stream_shuffle(rsb[:], sb[:], SWAP_MASK)
                        t1 = tmp.tile([128, 512], BF, name=f"t1_{tt}{h}{pj}", tag="t1")
                        nc.vector.tensor_tensor(t1[:], sb[:], cosT[:, tsl], ALU.mult)
                        t2 = tmp.tile([128, 512], BF, name=f"t2_{tt}{h}{pj}", tag="t2")
                        nc.vector.tensor_tensor(t2[:], rsb[:], sinT[:, tsl], ALU.mult)
                        nc.vector.tensor_tensor(dst[:, tsl], t1[:], t2[:], ALU.add)

            def emit_v(ts, s, sl, xs, ps_pool, ps_tag):
                # v for token-slice ts (128 tokens) from the cached x slab:
                # stationary = x^T column chunk, moving = wv slab chunk.
                jj = ts % 4
                vps = ps_pool.tile([128, 512], F32, name=f"vp{ts}", tag=ps_tag)
                for kc in range(NKC):
                    nc.tensor.matmul(vps[:], xs[:, kc * 512 + jj * 128: kc * 512 + (jj + 1) * 128],
                                     sl["wv"][:, kc * 512:(kc + 1) * 512],
                                     start=(kc == 0), stop=(kc == NKC - 1))
                nc.vector.tensor_tensor(v_sb[ts][:], vps[:], bv_t[:, s * 512:(s + 1) * 512],
                                        ALU.add)

            if do_a or do_b:
                # Prologue: stream-0 slabs + x(tt=0), chunk-interleaved in
                # needed-first order; cos/sin after the first chunk group.
                slabs = {0: new_slabs(0)}
                xs = new_xslab(0)
                for c in range(4):
                    if do_a:
                        chunk(slabs[0]["wq"], dts["wq"][0], c)
                        chunk(slabs[0]["wk"], dts["wk"][0], c)
                    chunk(xs, dts["xT"][0], c)
                    if c == 0:
                        nc.sync.dma_start(cosT[:], cos_d[:])
                        nc.sync.dma_start(sinT[:], sin_d[:])
                        nc.sync.dma_start(bias_t[:], bias_d[:])
                        nc.sync.dma_start(bv_t[:], bv_d[:])
                    elif do_b:
                        chunk(slabs[0]["wv"], dts["wv"][0], c - 1)
                if do_b:
                    chunk(slabs[0]["wv"], dts["wv"][0], 3)

                for tt in range(NTT):
                    s = tt // 2
                    if tt > 0:
                        xs = new_xslab(tt)
                        for c in range(4):
                            chunk(xs, dts["xT"][tt], c)
                    if tt == 1:
                        slabs[1] = new_slabs(1)
                        for c in range(4):
                            for nm in slabs[1]:
                                chunk(slabs[1][nm], dts[nm][1], c)
                    sl = slabs[s]
                    # At the last token tile, defer two v units into attention
                    # qt=0 (fills its exp-paced gaps) but keep two here so the
                    # PE stays busy while the final rope chains drain.
                    ndefer = 2 if (do_b and tt == NTT - 1 and "C" in phases) else 0
                    if do_a:
                        qps0, kps0 = emit_pair_mms(0, tt, sl, xs)
                        emit_rope(0, tt, s, qps0, kps0)
                        qps1, kps1 = emit_pair_mms(1, tt, sl, xs)
                        emit_rope(1, tt, s, qps1, kps1)
                    if do_b:
                        # v reuses pair-0's qk psum banks once the rope bias-
                        # activations have drained them
                        vtags = ("qk0q", "qk0k", "qk1q", "qk1k")
                        for u in range(4 - ndefer):
                            emit_v(4 * tt + u, s, sl, xs, qk_ps, vtags[u])
                        for u in range(4 - ndefer, 4):
                            deferred_v.append((4 * tt + u, s, sl, xs, emit_v))

        # ------- Phase C+D: attention + output projection, interleaved -----
        with ExitStack() as att:
            sps_ps = att.enter_context(tc.tile_pool(name="spsps", bufs=2, space="PSUM"))
            oacc_ps = att.enter_context(tc.tile_pool(name="oaccps", bufs=1, space="PSUM"))
            sums_ps_pool = att.enter_context(tc.tile_pool(name="sumsps", bufs=1, space="PSUM"))
            out_ps = att.enter_context(tc.tile_pool(name="outps", bufs=2, space="PSUM"))
            es_pool = att.enter_context(tc.tile_pool(name="es", bufs=4))
            sacc_pool = att.enter_context(tc.tile_pool(name="sacc", bufs=2))
            rc_pool = att.enter_context(tc.tile_pool(name="rc", bufs=2))
            wo_pool = att.enter_context(tc.tile_pool(name="wopool", bufs=2))
            osb_pool = att.enter_context(tc.tile_pool(name="osb", bufs=4))
            onorm_pool = att.enter_context(tc.tile_pool(name="onorm", bufs=1))

            o_norm = [onorm_pool.tile([128, T], BF, name=f"onorm{h}", tag=f"onorm{h}")
                      for h in range(HPC)]
            wo_slabs = {}
            osb_cur = [None]

            def issue_wo(s):
                wo_slab = wo_pool.tile([128, HPC * D], BF, name=f"wos{s}", tag="wo_slab")
                for c in range(4):
                    nc.sync.dma_start(wo_slab[:, c * CH:(c + 1) * CH],
                                      wo_d[s][:, c * CH:(c + 1) * CH])
                wo_slabs[s] = wo_slab

            def emit_outproj_chunk(qt, od, tail=False):
                qsl = slice(qt * 512, (qt + 1) * 512)
                wo_slab = wo_slabs[qt // 2]
                ops_t = out_ps.tile([128, 512], F32, name=f"op{qt}{od}", tag="o")
                for hd in range(HPC):
                    nc.tensor.matmul(
                        ops_t[:], wo_slab[:, hd * D + od * 128: hd * D + (od + 1) * 128],
                        o_norm[hd][:, qsl], start=(hd == 0), stop=(hd == HPC - 1))
                if od % 2 == 0:
                    osb_cur[0] = osb_pool.tile([128, 1024], BF, name=f"ou{qt}{od}", tag="osb")
                half = od % 2
                if tail:
                    # the final query tile drains after exp() is done — use the
                    # idle scalar engine so the DVE doesn't become the tail
                    # bottleneck
                    nc.scalar.activation(osb_cur[0][:, half * 512:(half + 1) * 512],
                                         ops_t[:], AF.Copy)
                else:
                    nc.vector.tensor_copy(osb_cur[0][:, half * 512:(half + 1) * 512], ops_t[:])
                if od % 2 == 1:
                    nc.sync.dma_start(out_d[qt][:, (od - 1) * 512:(od + 1) * 512], osb_cur[0][:])

            if "C" in phases:
                do_d = "D" in phases
                if do_d:
                    issue_wo(0)
                for qt in range(NTT):
                    qsl = slice(qt * 512, (qt + 1) * 512)
                    if qt == 2 and do_d:
                        issue_wo(1)
                    for h in range(HPC):
                        oacc = oacc_ps.tile([128, 512], F32, name=f"oa{h}{qt}", tag="oacc")
                        sacc = sacc_pool.tile([128, 1024], BF, name=f"sa{h}{qt}", tag="sacc")
                        es_prev = None
                        for j in range(NKC // 2):
                            sps = sps_ps.tile([128, 1024], F32, name=f"sp{h}{qt}{j}", tag="sps")
                            for half in range(2):
                                kc = 2 * j + half
                                nc.tensor.matmul(sps[:, half * 512:(half + 1) * 512],
                                                 k_rope[h][:, kc * 128:(kc + 1) * 128],
                                                 q_rope[h][:, qsl], start=True, stop=True)
                            es = es_pool.tile([128, 1024], BF, name=f"es{h}{qt}{j}", tag="es")
                            nc.scalar.activation(es[:], sps[:], AF.Exp, bias=zero_t[:, 0:1])
                            for half in range(2):
                                kc = 2 * j + half
                                nc.tensor.matmul(oacc[:], v_sb[kc][:, h * DH:(h + 1) * DH],
                                                 es[:, half * 512:(half + 1) * 512],
                                                 start=(kc == 0), stop=(kc == NKC - 1))
                            if j == 1:
                                nc.vector.tensor_tensor(sacc[:], es_prev[:], es[:], ALU.add)
                            elif j > 1:
                                nc.vector.tensor_tensor(sacc[:], sacc[:], es[:], ALU.add)
                            es_prev = es
                            # Interleave one ready output-projection chunk of the
                            # previous query tile after every even j (4 per head x
                            # 4 heads = all 16 chunks of qt-1); even j keeps the
                            # chunk's DVE copy out of the queue right before the
                            # denominator chain that gates the next head's PV.
                            # At qt=0 there is no projection work yet — the
                            # deferred last-token-tile v units fill the exp-paced
                            # PE gaps instead (they must land before j=6 consumes
                            # v_sb[14:]).
                            if do_d and qt > 0 and j % 2 == 1:
                                emit_outproj_chunk(qt - 1, h * 4 + j // 2)
                            if qt == 0 and h == 0 and j < len(deferred_v):
                                ts, s_, sl_, xs_, ev = deferred_v[j]
                                ev(ts, s_, sl_, xs_, out_ps, "o")
                        sfold = sacc_pool.tile([128, 512], BF, name=f"sf{h}{qt}", tag="sfold")
                        nc.vector.tensor_tensor(sfold[:], sacc[:, 0:512], sacc[:, 512:1024], ALU.add)
                        sums = sums_ps_pool.tile([128, 512], F32, name=f"su{h}{qt}", tag="sums")
                        nc.tensor.matmul(sums[:], ones_t[:], sfold[:], start=True, stop=True)
                        rc = rc_pool.tile([128, 512], F32, name=f"rc{h}{qt}", tag="rc")
                        nc.vector.reciprocal(rc[:], sums[:])
                        nc.vector.tensor_tensor(o_norm[h][:, qsl], oacc[:], rc[:], ALU.mult)
                if do_d:
                    for od in range(NKC):
                        emit_outproj_chunk(NTT - 1, od, tail=True)


def shard_inputs(inputs):
    """Full inputs -> per-core in_maps (all host-side prep: transpose, cast,
    scale-folding, per-head slicing, partition-major repacks)."""
    f32 = np.float32
    x1, x2 = np.asarray(inputs["x_1"], f32), np.asarray(inputs["x_2"], f32)
    cosT = np.ascontiguousarray(
        np.concatenate([np.asarray(inputs["cos1"]), np.asarray(inputs["cos2"])], 0).T
    ).astype(bf16)
    sinT = np.concatenate([np.asarray(inputs["sin1"]), np.asarray(inputs["sin2"])], 0).T.copy()
    sinT[0::2, :] *= -1.0      # fold the pair-rotation signs into sin
    sinT = np.ascontiguousarray(sinT).astype(bf16)

    in_maps = []
    for c in range(N_CORES):
        b, hg = divmod(c, 4)
        hsl = slice(hg * HPC * DH, (hg + 1) * HPC * DH)
        xc = np.concatenate([x1[b], x2[b]], 0)          # [T, D]
        xT = xc.T.reshape(NKC, 128, NTT, 512)
        xTT = np.ascontiguousarray(xT.transpose(2, 1, 0, 3).reshape(NTT, 128, SLAB)).astype(bf16)

        def wslice(name, scale=1.0):
            out = np.empty((2, 128, SLAB), bf16)
            for s in range(2):
                w = np.asarray(inputs[name + str(s + 1)], f32)[:, hsl] * scale
                out[s] = w.reshape(NKC, 128, HPC * DH).transpose(1, 0, 2).reshape(128, SLAB).astype(bf16)
            return out

        wq = wslice("wq", SCALE)
        wk = wslice("wk")
        wv = wslice("wv")
        wo = np.empty((2, 128, HPC * D), bf16)
        for s in range(2):
            w = np.asarray(inputs["wo" + str(s + 1)], f32)[hsl, :]
            wo[s] = w.reshape(HPC, 128, D).transpose(1, 0, 2).reshape(128, HPC * D).astype(bf16)

        bias = np.zeros((128, 16), f32)
        for s in range(2):
            bqs = np.asarray(inputs["bq" + str(s + 1)], f32)[hsl] * SCALE
            bks = np.asarray(inputs["bk" + str(s + 1)], f32)[hsl]
            for h in range(HPC):
                bias[:, s * 8 + h] = bqs[h * DH:(h + 1) * DH]
                bias[:, s * 8 + 4 + h] = bks[h * DH:(h + 1) * DH]
        bv = np.concatenate([
            np.asarray(inputs["bv1"], f32)[hsl], np.asarray(inputs["bv2"], f32)[hsl]
        ]).reshape(1, 2 * HPC * DH)
        bv = np.ascontiguousarray(np.broadcast_to(bv, (128, 2 * HPC * DH))).astype(bf16)

        in_maps.append({
            "xT": xTT, "wq": wq, "wk": wk, "wv": wv, "wo": wo,
            "bias_qk": bias, "bv": bv, "cosT": cosT, "sinT": sinT,
        })
    return in_maps


def unshard_outputs(results, inputs):
    f32 = np.float32
    acc = np.zeros((B, D, T), f32)
    for c in range(N_CORES):
        r = results[c]["outT"].astype(f32)               # [NTT, 128, NKC*512]
        acc[c // 4] += r.reshape(NTT, 128, NKC, 512).transpose(2, 1, 0, 3).reshape(D, T)
    o1 = np.empty((B, N1, D), f32)
    o2 = np.empty((B, N2, D), f32)
    bo1 = np.asarray(inputs["bo1"], f32)
    bo2 = np.asarray(inputs["bo2"], f32)
    for b in range(B):
        full = acc[b].T                                  # [T, D]
        o1[b] = full[:N1] + bo1
        o2[b] = full[N1:] + bo2
    return o1, o2


def kernel(**inputs):
    nc = build_program()
    in_maps = shard_inputs(inputs)
    res = run_bass_kernel_spmd(nc, in_maps, list(range(N_CORES)))
    return unshard_outputs(res.results, inputs)


if __name__ == "__main__":
    data = np.load("/root/problem/cache_inputs.npz")
    out = kernel(**{k: data[k] for k in data.files})
    exp = np.load("/root/problem/cache_expected.npz")
    for i, o in enumerate(out):
        e = exp[f"o{i+1}"]
        d = np.abs(o - e).max()
        print(f"o{i+1}: absmax_err {d:.4e} rel {d / np.abs(e).max():.4e}")


# revision 39
# speedup vs baseline: 1.8633x; 1.3936x over previous
"""Dual-stream multi-head attention on 8 Trainium2 NeuronCores (Bass/Tile).

Sharding: core c handles batch b = c//4 and head-group g = c%4 (4 of 16 heads).
Each core computes QKV projections (per-stream weights), RoPE, joint attention
over both streams, and a partial output projection (its heads' rows of wo).
The host sums the 4 per-core partials of each batch, transposes, and adds the
output bias.

On-chip layout is fully transposed ("feature dim on partitions, tokens on the
free dim"): x^T, q^T, k^T are [d, tokens]; scores are computed directly as
S^T = k_rope @ q_rope^T (k-positions on partitions), which lets the PV matmul
consume exp(S^T) with v in natural [token, dh] layout and produce o^T — the
exact layout the output projection wants.

Schedule notes (v3):
- All bulk tensors are stored partition-major in DRAM so DMAs move 2-16KB
  contiguous runs per partition (descriptor-efficient), chunked 4-ways so the
  PE can start consuming while the rest streams in.  Weight slabs and x tiles
  prefetch ahead of use (stream-1 slabs during tt=1, wo slabs in phase C).
- x^T is DMA'd ONCE per 512-token tile as a [128, 8192] slab shared by both
  head-pair QK passes and the V matmuls (stationary [128,128] column slices).
- RoPE's pair rotation runs on the DVE as a stream_shuffle (partition pair
  swap) with the signs folded into the host-prepared sin table; cos/sin
  multiplies and the combine run in bf16 (DVE 2x mode).  No PE or PSUM
  involvement beyond the projection itself.
- v bias is added by the DVE during the PSUM->SBUF move (host-replicated
  bias tile), freeing the scalar engine and the PE rank-1 matmul.
- exp() runs on [128,1024] tiles (two k-chunks per activation) to amortize
  activation-engine overhead; softmax denominators are chunk-accumulated in
  bf16 on the DVE (2x mode), folded, then row-summed with one ones-matmul.
- Output-projection chunks for query-tile qt-1 are interleaved into the
  attention j-loop of qt so the in-order PE queue always has ready matmuls
  while exp() catches up; out partials leave as bf16 pairs ([128,1024] DMAs)
  and the host sums partials in f32.
"""

import sys
import numpy as np

sys.path.insert(0, "/opt/trn_rl_repo")

import ml_dtypes
import concourse.bass as bass
import concourse.mybir as mybir
import concourse.tile as tile
from concourse.bass_utils import run_bass_kernel_spmd
from contextlib import ExitStack

B, N1, N2, D, H = 2, 1024, 1024, 2048, 16
T = N1 + N2              # 2048 tokens (both streams, concatenated)
DH = D // H              # 128
HPC = 4                  # heads per core
NKC = D // 128           # 16 contraction chunks
NTT = T // 512           # 4 512-token tiles
NTS = T // 128           # 16 128-token tiles
SCALE = DH ** -0.5
N_CORES = 8
SLAB = NKC * 512         # 8192 columns in a weight/x slab
CH = SLAB // 4           # 2048-column DMA chunks

BF = mybir.dt.bfloat16
F32 = mybir.dt.float32
F8 = mybir.dt.float8e4
bf16 = ml_dtypes.bfloat16
f8e4 = ml_dtypes.float8_e4m3
AF = mybir.ActivationFunctionType
ALU = mybir.AluOpType
DR = mybir.MatmulPerfMode.DoubleRow
SWAP_MASK = [i ^ 1 for i in range(32)]   # partition pair swap
WS = 32.0   # fp8 weight pre-scale: lifts the ~0.02-scale weights out of
            # e4m3's subnormal range; undone in the PSUM->SBUF activations

_BUILT = {}  # (repeats, phases) -> nc cache — build each program variant once


def build_program(repeats=1, phases="ABCD", variant=""):
    """variant flags (comma-set): 'vpsown' v uses its own PSUM pool;
    'nodefer' disable last-tile v deferral; 'v3order' phase-A order
    pair0,v0,pair1,rope0,v1,v2,rope1,v3."""
    global _BUILT
    key = (repeats, phases, variant)
    if key in _BUILT:
        return _BUILT[key]

    nc = bass.Bass()

    dts = {}
    dts["xT"] = nc.dram_tensor("xT", [NTT, 128, SLAB], BF, kind="ExternalInput")
    for nm in ("wq", "wk", "wv"):
        dts[nm] = nc.dram_tensor(nm, [2, 128, SLAB], BF, kind="ExternalInput")
    dts["wo"] = nc.dram_tensor("wo", [2, 128, HPC * D], BF, kind="ExternalInput")
    dts["bias_qk"] = nc.dram_tensor("bias_qk", [128, 16], F32, kind="ExternalInput")
    dts["bv"] = nc.dram_tensor("bv", [128, 2 * HPC * DH], BF, kind="ExternalInput")
    dts["cosT"] = nc.dram_tensor("cosT", [128, T], BF, kind="ExternalInput")
    dts["sinT"] = nc.dram_tensor("sinT", [128, T], BF, kind="ExternalInput")
    dts["outT"] = nc.dram_tensor("outT", [NTT, 128, SLAB], BF, kind="ExternalOutput")

    with tile.TileContext(nc) as tc:
        for _ in range(repeats):
            _emit(tc, nc, dts, phases=phases, variant=set(variant.split(",")))

    _split_dma_waits(nc)
    _BUILT[key] = nc
    return nc


def _split_dma_waits(nc):
    """This walrus build's 64-byte instruction encoding holds exactly one sync
    wait and it does not auto-split ("Too many sync wait commands") when Tile
    assigns two or more.  Peel the extras into standalone EventSemaphore waits
    on the same engine immediately before the instruction."""
    wid = 0
    fn = nc.m.functions[0]
    for blk in fn.blocks:
        insts = blk.instructions
        out = []
        changed = False
        for inst in insts:
            si = inst.sync_info
            if si is not None and len(si.on_wait) > 1:
                waits = list(si.on_wait)
                for w in waits[:-1]:
                    pre = mybir.InstEventSemaphore(
                        name=f"WSPLIT-{wid}", ins=[], outs=[])
                    wid += 1
                    pre.engine = inst.engine
                    pre.sync_info = mybir.SyncInfo(on_wait=[w], on_update=[])
                    nc.register_instruction(pre, overwrite=True)
                    out.append(pre)
                inst.sync_info = mybir.SyncInfo(
                    on_wait=[waits[-1]], on_update=list(si.on_update))
                changed = True
            out.append(inst)
        if changed:
            blk.instructions = out


def _emit(tc, nc, dts, phases="ABCD", variant=frozenset()):
    wo_d, bias_d, bv_d = dts["wo"], dts["bias_qk"], dts["bv"]
    cos_d, sin_d, out_d = dts["cosT"], dts["sinT"], dts["outT"]
    with ExitStack() as top:
        consts = top.enter_context(tc.tile_pool(name="consts", bufs=1))
        persist = top.enter_context(tc.tile_pool(name="persist", bufs=1))

        bias_t = consts.tile([128, 16], F32, name="bias_t", tag="bias_t")
        bv_t = consts.tile([128, 2 * HPC * DH], BF, name="bv_t", tag="bv_t")
        cosT = consts.tile([128, T], BF, name="cosT_t", tag="cosT_t")
        sinT = consts.tile([128, T], BF, name="sinT_t", tag="sinT_t")
        ones_t = consts.tile([128, 128], BF, name="ones_t", tag="ones_t")
        nc.vector.memset(ones_t[:], 1.0)
        zero_t = consts.tile([128, 1], F32, name="zero_t", tag="zero_t")
        nc.vector.memset(zero_t[:], 0.0)

        q_rope = [persist.tile([128, T], BF, name=f"qrope{h}", tag=f"qrope{h}") for h in range(HPC)]
        k_rope = [persist.tile([128, T], BF, name=f"krope{h}", tag=f"krope{h}") for h in range(HPC)]
        v_sb = [persist.tile([128, HPC * DH], BF, name=f"vsb{ts}", tag=f"vsb{ts}") for ts in range(NTS)]

        # wv + x slabs live in the top scope: the v matmuls of the last token
        # tile are deferred into attention qt=0 to fill its exp-paced PE gaps.
        wv_pool = top.enter_context(tc.tile_pool(name="wvslab", bufs=2))
        xs_pool = top.enter_context(tc.tile_pool(name="xs", bufs=2))
        deferred_v = []          # (ts, s, slab-dict, xh, xl) emitted in phase C

        # ---------------- Phase A: q^T,k^T projections + RoPE --------------
        # ---------------- Phase B: v (natural layout) ----------------------
        with ExitStack() as ab:
            wslab = ab.enter_context(tc.tile_pool(name="wslab", bufs=2))
            tmp = ab.enter_context(tc.tile_pool(name="tmpab", bufs=3))
            qk_ps = ab.enter_context(tc.tile_pool(name="qkps", bufs=1, space="PSUM"))
            v_ps = (ab.enter_context(tc.tile_pool(name="vps", bufs=2, space="PSUM"))
                    if "vpsown" in variant else None)

            do_a = "A" in phases
            do_b = "B" in phases

            def chunk(dst, src, c):
                nc.sync.dma_start(dst[:, c * CH:(c + 1) * CH], src[:, c * CH:(c + 1) * CH])

            def new_slabs(s):
                out = {}
                if do_a:
                    for nm in ("wq", "wk"):
                        out[nm] = wslab.tile([128, SLAB], BF, name=f"{nm}{s}", tag=f"{nm}_slab")
                if do_b:
                    out["wv"] = wv_pool.tile([128, SLAB], BF, name=f"wv{s}", tag="wv_slab")
                return out

            def new_xslab(tt):
                return xs_pool.tile([128, SLAB], BF, name=f"x{tt}", tag="xslab")

            def emit_pair_mms(pair, tt, sl, xs):
                """One head pair's q,k projections, kc-major (DMA arrival
                order at tt=0)."""
                hs = (2 * pair, 2 * pair + 1)
                qps, kps = {}, {}
                for h in hs:
                    ht = h % 2 if v_ps is not None else h
                    qps[h] = qk_ps.tile([128, 512], F32, name=f"qps{tt}_{h}", tag=f"qk{ht}q")
                    kps[h] = qk_ps.tile([128, 512], F32, name=f"kps{tt}_{h}", tag=f"qk{ht}k")
                for kc in range(NKC):
                    xsl = xs[:, kc * 512:(kc + 1) * 512]
                    for h in hs:
                        wsl = slice(kc * 512 + h * DH, kc * 512 + (h + 1) * DH)
                        nc.tensor.matmul(qps[h][:], sl["wq"][:, wsl], xsl,
                                         start=(kc == 0), stop=(kc == NKC - 1))
                        nc.tensor.matmul(kps[h][:], sl["wk"][:, wsl], xsl,
                                         start=(kc == 0), stop=(kc == NKC - 1))
                return qps, kps

            def emit_rope(pair, tt, s, qps, kps):
                tsl = slice(tt * 512, (tt + 1) * 512)
                hs = (2 * pair, 2 * pair + 1)
                for h in hs:
                    for pj, (ps, dst) in enumerate(((qps[h], q_rope[h]), (kps[h], k_rope[h]))):
                        bj = s * 8 + pj * 4 + h
                        sb = tmp.tile([128, 512], BF, name=f"sb{tt}{h}{pj}", tag="psb")
                        nc.scalar.activation(sb[:], ps[:], AF.Identity,
                                             bias=bias_t[:, bj:bj + 1])
                        rsb = tmp.tile([128, 512], BF, name=f"rs{tt}{h}{pj}", tag="rsb")
                        nc.vector.stream_shuffle(rsb[:], sb[:], SWAP_MASK)
                        t1 = tmp.tile([128, 512], BF, name=f"t1_{tt}{h}{pj}", tag="t1")
                        nc.vector.tensor_tensor(t1[:], sb[:], cosT[:, tsl], ALU.mult)
                        t2 = tmp.tile([128, 512], BF, name=f"t2_{tt}{h}{pj}", tag="t2")
                        nc.vector.tensor_tensor(t2[:], rsb[:], sinT[:, tsl], ALU.mult)
                        nc.vector.tensor_tensor(dst[:, tsl], t1[:], t2[:], ALU.add)

            def emit_v(ts, s, sl, xs, ps_pool, ps_tag):
                # v for token-slice ts (128 tokens) from the cached x slab:
                # stationary = x^T column chunk, moving = wv slab chunk.
                jj = ts % 4
                vps = ps_pool.tile([128, 512], F32, name=f"vp{ts}", tag=ps_tag)
                for kc in range(NKC):
                    nc.tensor.matmul(vps[:], xs[:, kc * 512 + jj * 128: kc * 512 + (jj + 1) * 128],
                                     sl["wv"][:, kc * 512:(kc + 1) * 512],
                                     start=(kc == 0), stop=(kc == NKC - 1))
                nc.vector.tensor_tensor(v_sb[ts][:], vps[:], bv_t[:, s * 512:(s + 1) * 512],
                                        ALU.add)

            if do_a or do_b:
                # Prologue: stream-0 slabs + x(tt=0), chunk-interleaved in
                # needed-first order; cos/sin after the first chunk group.
                slabs = {0: new_slabs(0)}
                xs = new_xslab(0)
                for c in range(4):
                    if do_a:
                        chunk(slabs[0]["wq"], dts["wq"][0], c)
                        chunk(slabs[0]["wk"], dts["wk"][0], c)
                    chunk(xs, dts["xT"][0], c)
                    if c == 0:
                        nc.sync.dma_start(cosT[:], cos_d[:])
                        nc.sync.dma_start(sinT[:], sin_d[:])
                        nc.sync.dma_start(bias_t[:], bias_d[:])
                        nc.sync.dma_start(bv_t[:], bv_d[:])
                    elif do_b:
                        chunk(slabs[0]["wv"], dts["wv"][0], c - 1)
                if do_b:
                    chunk(slabs[0]["wv"], dts["wv"][0], 3)

                for tt in range(NTT):
                    s = tt // 2
                    if tt > 0:
                        xs = new_xslab(tt)
                        for c in range(4):
                            chunk(xs, dts["xT"][tt], c)
                    if tt == 1:
                        slabs[1] = new_slabs(1)
                        for c in range(4):
                            for nm in slabs[1]:
                                chunk(slabs[1][nm], dts[nm][1], c)
                    sl = slabs[s]
                    # At the last token tile, defer two v units into attention
                    # qt=0 (fills its exp-paced gaps) but keep two here so the
                    # PE stays busy while the final rope chains drain.
                    ndefer = (0 if "nodefer" in variant else 2) \
                        if (do_b and tt == NTT - 1 and "C" in phases) else 0
                    vpool = v_ps if v_ps is not None else qk_ps
                    vtags = (("vps",) * 4 if v_ps is not None
                             else ("qk0q", "qk0k", "qk1q", "qk1k"))

                    def do_v(u):
                        if u >= 4 - ndefer:
                            deferred_v.append((4 * tt + u, s, sl, xs, emit_v))
                        else:
                            emit_v(4 * tt + u, s, sl, xs, vpool, vtags[u])

                    if "v3order" in variant:
                        if do_a:
                            qps0, kps0 = emit_pair_mms(0, tt, sl, xs)
                        if do_b:
                            do_v(0)
                        if do_a:
                            qps1, kps1 = emit_pair_mms(1, tt, sl, xs)
                            emit_rope(0, tt, s, qps0, kps0)
                        if do_b:
                            do_v(1)
                            do_v(2)
                        if do_a:
                            emit_rope(1, tt, s, qps1, kps1)
                        if do_b:
                            do_v(3)
                    else:
                        if do_a:
                            qps0, kps0 = emit_pair_mms(0, tt, sl, xs)
                            emit_rope(0, tt, s, qps0, kps0)
                            qps1, kps1 = emit_pair_mms(1, tt, sl, xs)
                            emit_rope(1, tt, s, qps1, kps1)
                        if do_b:
                            for u in range(4):
                                do_v(u)

        # ------- Phase C+D: attention + output projection, interleaved -----
        with ExitStack() as att:
            sps_ps = att.enter_context(tc.tile_pool(name="spsps", bufs=2, space="PSUM"))
            oacc_ps = att.enter_context(tc.tile_pool(name="oaccps", bufs=1, space="PSUM"))
            sums_ps_pool = att.enter_context(tc.tile_pool(name="sumsps", bufs=1, space="PSUM"))
            out_ps = att.enter_context(tc.tile_pool(name="outps", bufs=2, space="PSUM"))
            es_pool = att.enter_context(tc.tile_pool(name="es", bufs=4))
            sacc_pool = att.enter_context(tc.tile_pool(name="sacc", bufs=2))
            rc_pool = att.enter_context(tc.tile_pool(name="rc", bufs=2))
            wo_pool = att.enter_context(tc.tile_pool(name="wopool", bufs=2))
            osb_pool = att.enter_context(tc.tile_pool(name="osb", bufs=4))
            onorm_pool = att.enter_context(tc.tile_pool(name="onorm", bufs=1))

            o_norm = [onorm_pool.tile([128, T], BF, name=f"onorm{h}", tag=f"onorm{h}")
                      for h in range(HPC)]
            wo_slabs = {}
            osb_cur = [None]

            def issue_wo(s):
                wo_slab = wo_pool.tile([128, HPC * D], BF, name=f"wos{s}", tag="wo_slab")
                for c in range(4):
                    nc.sync.dma_start(wo_slab[:, c * CH:(c + 1) * CH],
                                      wo_d[s][:, c * CH:(c + 1) * CH])
                wo_slabs[s] = wo_slab

            def emit_outproj_chunk(qt, od, tail=False):
                qsl = slice(qt * 512, (qt + 1) * 512)
                wo_slab = wo_slabs[qt // 2]
                ops_t = out_ps.tile([128, 512], F32, name=f"op{qt}{od}", tag="o")
                for hd in range(HPC):
                    nc.tensor.matmul(
                        ops_t[:], wo_slab[:, hd * D + od * 128: hd * D + (od + 1) * 128],
                        o_norm[hd][:, qsl], start=(hd == 0), stop=(hd == HPC - 1))
                if od % 2 == 0:
                    osb_cur[0] = osb_pool.tile([128, 1024], BF, name=f"ou{qt}{od}", tag="osb")
                half = od % 2
                if tail:
                    # the final query tile drains after exp() is done — use the
                    # idle scalar engine so the DVE doesn't become the tail
                    # bottleneck
                    nc.scalar.activation(osb_cur[0][:, half * 512:(half + 1) * 512],
                                         ops_t[:], AF.Copy)
                else:
                    nc.vector.tensor_copy(osb_cur[0][:, half * 512:(half + 1) * 512], ops_t[:])
                if od % 2 == 1:
                    nc.sync.dma_start(out_d[qt][:, (od - 1) * 512:(od + 1) * 512], osb_cur[0][:])

            if "C" in phases:
                do_d = "D" in phases
                if do_d:
                    issue_wo(0)
                for qt in range(NTT):
                    qsl = slice(qt * 512, (qt + 1) * 512)
                    if qt == 2 and do_d:
                        issue_wo(1)
                    for h in range(HPC):
                        oacc = oacc_ps.tile([128, 512], F32, name=f"oa{h}{qt}", tag="oacc")
                        sacc = sacc_pool.tile([128, 1024], BF, name=f"sa{h}{qt}", tag="sacc")
                        es_prev = None
                        for j in range(NKC // 2):
                            sps = sps_ps.tile([128, 1024], F32, name=f"sp{h}{qt}{j}", tag="sps")
                            for half in range(2):
                                kc = 2 * j + half
                                nc.tensor.matmul(sps[:, half * 512:(half + 1) * 512],
                                                 k_rope[h][:, kc * 128:(kc + 1) * 128],
                                                 q_rope[h][:, qsl], start=True, stop=True)
                            es = es_pool.tile([128, 1024], BF, name=f"es{h}{qt}{j}", tag="es")
                            nc.scalar.activation(es[:], sps[:], AF.Exp, bias=zero_t[:, 0:1])
                            for half in range(2):
                                kc = 2 * j + half
                                nc.tensor.matmul(oacc[:], v_sb[kc][:, h * DH:(h + 1) * DH],
                                                 es[:, half * 512:(half + 1) * 512],
                                                 start=(kc == 0), stop=(kc == NKC - 1))
                            if j == 1:
                                nc.vector.tensor_tensor(sacc[:], es_prev[:], es[:], ALU.add)
                            elif j > 1:
                                nc.vector.tensor_tensor(sacc[:], sacc[:], es[:], ALU.add)
                            es_prev = es
                            # Interleave one ready output-projection chunk of the
                            # previous query tile after every even j (4 per head x
                            # 4 heads = all 16 chunks of qt-1); even j keeps the
                            # chunk's DVE copy out of the queue right before the
                            # denominator chain that gates the next head's PV.
                            # At qt=0 there is no projection work yet — the
                            # deferred last-token-tile v units fill the exp-paced
                            # PE gaps instead (they must land before j=6 consumes
                            # v_sb[14:]).
                            if do_d and qt > 0 and j % 2 == 1:
                                emit_outproj_chunk(qt - 1, h * 4 + j // 2)
                            if qt == 0 and h == 0 and j < len(deferred_v):
                                ts, s_, sl_, xs_, ev = deferred_v[j]
                                ev(ts, s_, sl_, xs_, out_ps, "o")
                        sfold = sacc_pool.tile([128, 512], BF, name=f"sf{h}{qt}", tag="sfold")
                        nc.vector.tensor_tensor(sfold[:], sacc[:, 0:512], sacc[:, 512:1024], ALU.add)
                        sums = sums_ps_pool.tile([128, 512], F32, name=f"su{h}{qt}", tag="sums")
                        nc.tensor.matmul(sums[:], ones_t[:], sfold[:], start=True, stop=True)
                        rc = rc_pool.tile([128, 512], F32, name=f"rc{h}{qt}", tag="rc")
                        nc.vector.reciprocal(rc[:], sums[:])
                        nc.vector.tensor_tensor(o_norm[h][:, qsl], oacc[:], rc[:], ALU.mult)
                if do_d:
                    for od in range(NKC):
                        emit_outproj_chunk(NTT - 1, od, tail=True)


def shard_inputs(inputs):
    """Full inputs -> per-core in_maps (all host-side prep: transpose, cast,
    scale-folding, per-head slicing, partition-major repacks)."""
    f32 = np.float32
    x1, x2 = np.asarray(inputs["x_1"], f32), np.asarray(inputs["x_2"], f32)
    cosT = np.ascontiguousarray(
        np.concatenate([np.asarray(inputs["cos1"]), np.asarray(inputs["cos2"])], 0).T
    ).astype(bf16)
    sinT = np.concatenate([np.asarray(inputs["sin1"]), np.asarray(inputs["sin2"])], 0).T.copy()
    sinT[0::2, :] *= -1.0      # fold the pair-rotation signs into sin
    sinT = np.ascontiguousarray(sinT).astype(bf16)

    in_maps = []
    for c in range(N_CORES):
        b, hg = divmod(c, 4)
        hsl = slice(hg * HPC * DH, (hg + 1) * HPC * DH)
        xc = np.concatenate([x1[b], x2[b]], 0)          # [T, D]
        xT = xc.T.reshape(NKC, 128, NTT, 512)
        xTT = np.ascontiguousarray(xT.transpose(2, 1, 0, 3).reshape(NTT, 128, SLAB)).astype(bf16)

        def wslice(name, scale=1.0):
            out = np.empty((2, 128, SLAB), bf16)
            for s in range(2):
                w = np.asarray(inputs[name + str(s + 1)], f32)[:, hsl] * scale
                out[s] = w.reshape(NKC, 128, HPC * DH).transpose(1, 0, 2).reshape(128, SLAB).astype(bf16)
            return out

        wq = wslice("wq", SCALE)
        wk = wslice("wk")
        wv = wslice("wv")
        wo = np.empty((2, 128, HPC * D), bf16)
        for s in range(2):
            w = np.asarray(inputs["wo" + str(s + 1)], f32)[hsl, :]
            wo[s] = w.reshape(HPC, 128, D).transpose(1, 0, 2).reshape(128, HPC * D).astype(bf16)

        bias = np.zeros((128, 16), f32)
        for s in range(2):
            bqs = np.asarray(inputs["bq" + str(s + 1)], f32)[hsl] * SCALE
            bks = np.asarray(inputs["bk" + str(s + 1)], f32)[hsl]
            for h in range(HPC):
                bias[:, s * 8 + h] = bqs[h * DH:(h + 1) * DH]
                bias[:, s * 8 + 4 + h] = bks[h * DH:(h + 1) * DH]
        bv = np.concatenate([
            np.asarray(inputs["bv1"], f32)[hsl], np.asarray(inputs["bv2"], f32)[hsl]
        ]).reshape(1, 2 * HPC * DH)
        bv = np.ascontiguousarray(np.broadcast_to(bv, (128, 2 * HPC * DH))).astype(bf16)

        in_maps.append({
            "xT": xTT, "wq": wq, "wk": wk, "wv": wv, "wo": wo,
            "bias_qk": bias, "bv": bv, "cosT": cosT, "sinT": sinT,
        })
    return in_maps


def unshard_outputs(results, inputs):
    f32 = np.float32
    acc = np.zeros((B, D, T), f32)
    for c in range(N_CORES):
        r = results[c]["outT"].astype(f32)               # [NTT, 128, NKC*512]
        acc[c // 4] += r.reshape(NTT, 128, NKC, 512).transpose(2, 1, 0, 3).reshape(D, T)
    o1 = np.empty((B, N1, D), f32)
    o2 = np.empty((B, N2, D), f32)
    bo1 = np.asarray(inputs["bo1"], f32)
    bo2 = np.asarray(inputs["bo2"], f32)
    for b in range(B):
        full = acc[b].T                                  # [T, D]
        o1[b] = full[:N1] + bo1
        o2[b] = full[N1:] + bo2
    return o1, o2


def kernel(**inputs):
    nc = build_program()
    in_maps = shard_inputs(inputs)
    res = run_bass_kernel_spmd(nc, in_maps, list(range(N_CORES)))
    return unshard_outputs(res.results, inputs)


if __name__ == "__main__":
    data = np.load("/root/problem/cache_inputs.npz")
    out = kernel(**{k: data[k] for k in data.files})
    exp = np.load("/root/problem/cache_expected.npz")
    for i, o in enumerate(out):
        e = exp[f"o{i+1}"]
        d = np.abs(o - e).max()
        print(f"o{i+1}: absmax_err {d:.4e} rel {d / np.abs(e).max():.4e}")
